# revision 14
# baseline (speedup 1.0000x reference)
"""GCN layer (2 edge types, mean aggregation + self-loop) on 8 Trainium2 cores.

Math (per reference):
    m_t = segment_mean(h[src_t] @ Wt.T, dst_t)   for t in {1,2}
    out = relu(h @ Wl.T + bl + 0.5*(m1 + m2))

Linear commutes with gather+mean: raw h rows are segment-mean'd first and
the 128x128 weights applied afterwards.

Design (v3) — measured bottlenecks drive everything:
  * dma_gather costs ~8ns/descriptor (row) regardless of row size, and
    4 SWDGE queues run near-parallel -> minimize descriptors, spread
    calls round-robin over 4 queues, gather single-bf16 rows (256B).
  * dst nodes partitioned contiguously across 8 cores (12500 = 98 slots
    of 128).  Slots processed in 9 PSUM-resident groups (8x12 + 2);
    each (type, quad-of-4-slots) owns one PSUM bank ([128, 512] f32)
    alive across all 4 src windows -> no SBUF accumulator traffic.
    Only the bank's globally-first matmul sets start (a start clears
    accumulate-bits bank-wide), only its last sets stop.
  * Segment-sum as a flipped indicator matmul
        psT[f, d] += sum_e g[e, f] * ind[e, d]
    (matmul(lhsT=g_chunk, rhs=ind)) giving the transposed mean directly.
    ind = is_equal(iota, drel) on DVE (unfused: a fused second ALU op
    measures 2x slower).  Chunks spanning two adjacent slots of a quad
    use one 256-wide indicator instead of two 128-wide ones.
  * The 1/deg mean scale is applied on the PSUM->SBUF move: one DVE
    tensor_tensor mult per (type, slot-pair) against a DMA-loaded
    partition-replicated inv table.
  * Edges routed to (core, type, group, window) cells, packed densely
    into chunks of 128 (93% fill); src indices int16 relative to one of
    4 windows of 25000 rows.
  * Final per slot-pair: 3 float32r matmuls (256-wide, full PE rate):
        out.T = relu(0.5*W1.T' m1T + 0.5*W2.T' m2T + Wl.T' hT + bl)

All 8 cores share one SPMD instruction stream; the chunk/subchunk
schedule is the max-shape over cores, per-core tables (gather indices,
drel scalar columns, inv tables) specialize it.  Padding slots gather
window row 0 and carry a sentinel drel -> indicator 0.
"""

import numpy as np
import ml_dtypes

BF16 = np.dtype(ml_dtypes.bfloat16)

# ---------------------------------------------------------------- config ---

N_NODES = 100000
HIDDEN = 128
N_CORES = 8
ROWS_PER_CORE = N_NODES // N_CORES  # 12500
S = 98                        # dst slots per core (12544 >= 12500)
GROUP_SIZES = [12] * 8 + [2]  # PSUM-resident slot groups
NW = 4                        # src windows
WBASE = 25000                 # window w covers rows [w*WBASE, (w+1)*WBASE)
KG = 4                        # chunks per dma_gather call
NQ = 4                        # SWDGE queues
SCRATCH = 32768               # dynamic DMA descriptor carveout
SENT = 999.0                  # drel sentinel -> indicator 0


def _cdiv(a, b):
    return -(-a // b)


# ------------------------------------------------------------ host routing ---

def _route(srcs, dsts):
    """Build the shared (static) chunk/subchunk schedule + per-core tables."""
    NG = len(GROUP_SIZES)
    grp_base = np.concatenate([[0], np.cumsum(GROUP_SIZES)[:-1]])
    grp_of = np.repeat(np.arange(NG), GROUP_SIZES)  # slot -> group

    n_types = len(srcs)
    invdeg = []
    for t in range(n_types):
        deg = np.bincount(dsts[t].astype(np.int64), minlength=N_NODES)
        invdeg.append((1.0 / np.maximum(deg, 1)).astype(np.float32))

    ed = []
    for t in range(n_types):
        src = srcs[t].astype(np.int64)
        dst = dsts[t].astype(np.int64)
        c = dst // ROWS_PER_CORE
        dl = dst - c * ROWS_PER_CORE
        s = dl >> 7
        d128 = (dl & 127).astype(np.float32)
        g = grp_of[s]
        w = src // WBASE
        idx16 = (src - w * WBASE).astype(np.int16)
        order = np.lexsort((src, s, w, g, c))
        ed.append(dict(c=c[order], s=s[order], d128=d128[order],
                       g=g[order], w=w[order], idx16=idx16[order]))

    gmax = max(GROUP_SIZES)
    cnt = np.zeros((n_types, N_CORES, NG, NW), np.int64)
    cnt_s = np.zeros((n_types, N_CORES, NG, NW, gmax), np.int64)
    for t in range(n_types):
        e = ed[t]
        s_loc = e["s"] - grp_base[e["g"]]
        np.add.at(cnt[t], (e["c"], e["g"], e["w"]), 1)
        np.add.at(cnt_s[t], (e["c"], e["g"], e["w"], s_loc), 1)

    caps = np.zeros((NG, NW, n_types), np.int64)
    for g in range(NG):
        for w in range(NW):
            for t in range(n_types):
                caps[g, w, t] = _cdiv(int(cnt[t][:, g, w].max()), 128)
    for g in range(NG):
        for t in range(n_types):
            if caps[g, :, t].sum() == 0:
                caps[g, 0, t] = 1

    chunk_base = np.zeros((NG, NW, n_types), np.int64)
    pos = 0
    for g in range(NG):
        for w in range(NW):
            for t in range(n_types):
                chunk_base[g, w, t] = pos
                pos += int(caps[g, w, t])
    n_chunks = pos

    calls = []  # (window, col0, width)
    for g in range(NG):
        for w in range(NW):
            c0 = int(chunk_base[g, w, 0])
            c1 = int(chunk_base[g, w, n_types - 1] + caps[g, w, n_types - 1])
            c = c0
            while c < c1:
                wd = min(KG, c1 - c)
                calls.append((w, c, wd))
                c += wd

    # per-chunk union (over cores) of spanned local slots
    slots_of_chunk = [set() for _ in range(n_chunks)]
    for g in range(NG):
        gsz = GROUP_SIZES[g]
        for w in range(NW):
            for t in range(n_types):
                Q = int(caps[g, w, t])
                if Q == 0:
                    continue
                base = int(chunk_base[g, w, t])
                for c in range(N_CORES):
                    cum = 0
                    for sl in range(gsz):
                        n = int(cnt_s[t][c, g, w, sl])
                        if n == 0:
                            continue
                        q0, q1 = cum // 128, (cum + n - 1) // 128
                        for q in range(q0, q1 + 1):
                            slots_of_chunk[base + q].add(sl)
                        cum += n

    # coverage injection for (t, s) with no edges anywhere
    covered = np.zeros((n_types, S), bool)
    for g in range(NG):
        for w in range(NW):
            for t in range(n_types):
                base = int(chunk_base[g, w, t])
                for q in range(int(caps[g, w, t])):
                    for sl in slots_of_chunk[base + q]:
                        covered[t, grp_base[g] + sl] = True
    for t in range(n_types):
        for s in range(S):
            if not covered[t, s]:
                g = int(grp_of[s])
                for w in range(NW):
                    if caps[g, w, t] > 0:
                        base = int(chunk_base[g, w, t])
                        slots_of_chunk[base].add(s - int(grp_base[g]))
                        break

    # merge adjacent slots (within a quad) into 256-wide subchunks
    # subs_of_chunk[ci] = [(sl_lo, n_slots)]; cover[(ci, sl)] = (j, sl_lo)
    subs_of_chunk = [[] for _ in range(n_chunks)]
    for ci in range(n_chunks):
        SL = sorted(slots_of_chunk[ci])
        i = 0
        while i < len(SL):
            sl = SL[i]
            if (i + 1 < len(SL) and SL[i + 1] == sl + 1 and sl % 4 < 3):
                subs_of_chunk[ci].append((sl, 2))
                i += 2
            else:
                subs_of_chunk[ci].append((sl, 1))
                i += 1

    sub_id = {}     # (ci, sl_lo) -> j
    cover = {}      # (ci, sl) -> (j, sl_lo)
    chunk_cell = [None] * n_chunks
    first_q, last_q = {}, {}   # (t, g, quad) -> j
    j = 0
    for g in range(NG):
        for w in range(NW):
            for t in range(n_types):
                base = int(chunk_base[g, w, t])
                for q in range(int(caps[g, w, t])):
                    ci = base + q
                    chunk_cell[ci] = (g, w, t)
                    for (sl, ns) in subs_of_chunk[ci]:
                        sub_id[(ci, sl)] = j
                        for k in range(ns):
                            cover[(ci, sl + k)] = (j, sl)
                        qkey = (t, g, sl // 4)
                        if qkey not in first_q:
                            first_q[qkey] = j
                        last_q[qkey] = j
                        j += 1
    n_sub = j

    # ------------------------------------------------------ per-core tables
    per_core = []
    for c in range(N_CORES):
        flat_idx = np.zeros(n_chunks * 128, np.int16)
        scl = np.full((128, n_sub), SENT, np.float32)
        for t in range(n_types):
            e = ed[t]
            mask = e["c"] == c
            idx = np.nonzero(mask)[0]
            if len(idx) == 0:
                continue
            gs, ws = e["g"][idx], e["w"][idx]
            sl = e["s"][idx] - grp_base[gs]
            cellkey = gs * NW + ws
            bounds = np.nonzero(np.diff(cellkey))[0] + 1
            starts = np.concatenate([[0], bounds])
            ends = np.concatenate([bounds, [len(idx)]])
            for lo, hi in zip(starts, ends):
                g, w = int(gs[lo]), int(ws[lo])
                base = int(chunk_base[g, w, t])
                p = np.arange(hi - lo)
                eidx = idx[lo:hi]
                flat_idx[base * 128 + p] = e["idx16"][eidx]
                slr = sl[lo:hi]
                rb = np.nonzero(np.diff(slr))[0] + 1
                rst = np.concatenate([[0], rb])
                ren = np.concatenate([rb, [hi - lo]])
                for a, b in zip(rst, ren):
                    s_loc = int(slr[a])
                    for q in range(a // 128, (b - 1) // 128 + 1):
                        pa, pb = max(a, q * 128), min(b, (q + 1) * 128)
                        jj, sl_lo = cover[(base + q, s_loc)]
                        rows = np.arange(pa, pb) % 128
                        sel = eidx[pa:pb]
                        scl[rows, jj] = (e["d128"][sel]
                                         + 128.0 * (s_loc - sl_lo))

        gidx = np.zeros((128, n_chunks * 8), np.int16)
        for (w, col0, wd) in calls:
            seg = flat_idx[col0 * 128:(col0 + wd) * 128]
            gidx[:, col0 * 8:(col0 + wd) * 8] = \
                np.tile(seg.reshape(-1, 16).T, (8, 1))

        invb = []
        for t in range(n_types):
            row = np.zeros(S * 128, np.float32)
            row[:ROWS_PER_CORE] = invdeg[t][c * ROWS_PER_CORE:
                                            (c + 1) * ROWS_PER_CORE]
            invb.append(np.ascontiguousarray(
                np.broadcast_to(row, (128, S * 128))))
        per_core.append(dict(gidx=np.ascontiguousarray(gidx), scl=scl,
                             invb=invb))

    return dict(caps=caps, chunk_base=chunk_base, n_chunks=n_chunks,
                n_sub=n_sub, calls=calls, subs_of_chunk=subs_of_chunk,
                sub_id=sub_id, chunk_cell=chunk_cell,
                first_q=first_q, last_q=last_q,
                grp_base=grp_base, per_core=per_core)


# ------------------------------------------------------------ bass program ---

def _build_program(rt, n_nodes, n_cores, reps=1):
    import os
    import concourse.bacc as bacc
    from concourse import mybir, tile, library_config

    mode = os.environ.get("KMODE", "full")  # full | gather | noind | nomm
    NG = len(GROUP_SIZES)
    n_types = 2
    caps, chunk_base = rt["caps"], rt["chunk_base"]
    n_chunks, n_sub = rt["n_chunks"], rt["n_sub"]
    calls, subs_of_chunk = rt["calls"], rt["subs_of_chunk"]
    sub_id = rt["sub_id"]
    first_q, last_q = rt["first_q"], rt["last_q"]
    grp_base = rt["grp_base"]

    nc = bacc.Bacc("TRN2", target_bir_lowering=False, debug=False,
                   num_devices=n_cores, dynamic_dma_scratch_size=SCRATCH,
                   num_swdge_queues=NQ)
    dt = mybir.dt

    hpk = nc.dram_tensor("hpk", [n_nodes, 128], dt.bfloat16,
                         kind="ExternalInput").ap()
    gidx_d = nc.dram_tensor("gidx", [128, n_chunks * 8], dt.int16,
                            kind="ExternalInput").ap()
    scl_d = nc.dram_tensor("scl", [128, n_sub], dt.float32,
                           kind="ExternalInput").ap()
    invb_d = [nc.dram_tensor(f"invb{t}", [128, S * 128], dt.float32,
                             kind="ExternalInput").ap()
              for t in range(n_types)]
    hot_d = nc.dram_tensor("hot", [128, S * 128], dt.float32r,
                           kind="ExternalInput").ap()
    w_d = [nc.dram_tensor(w, [128, 128], dt.float32r,
                          kind="ExternalInput").ap()
           for w in ("w1t", "w2t", "wlt")]
    blc_d = nc.dram_tensor("blc", [128, 1], dt.float32,
                           kind="ExternalInput").ap()
    iota_d = nc.dram_tensor("iota", [128, 256], dt.bfloat16,
                            kind="ExternalInput").ap()
    outT_d = nc.dram_tensor("outT", [128, S * 128], dt.float32,
                            kind="ExternalOutput").ap()

    call_of_chunk = {}
    for k, (w, col0, wd) in enumerate(calls):
        for ci in range(col0, col0 + wd):
            call_of_chunk[ci] = k

    with tile.TileContext(nc) as tc:
        with (
            tc.tile_pool(name="const", bufs=1) as const_p,
            tc.tile_pool(name="gpool", bufs=6) as gpool,
            tc.tile_pool(name="ind", bufs=8) as ind_p,
            tc.tile_pool(name="mt", bufs=2) as mt_p,
            tc.tile_pool(name="invb", bufs=2) as invb_p,
            tc.tile_pool(name="hot", bufs=2) as hot_p,
            tc.tile_pool(name="ostage", bufs=2) as o_p,
            tc.tile_pool(name="psT", bufs=1, space="PSUM") as psT_p,
            tc.tile_pool(name="pso", bufs=2, space="PSUM") as pso_p,
        ):
            nc.gpsimd.load_library(library_config.mlp)
            gidx_s = const_p.tile([128, n_chunks * 8], dt.int16, name="gidx_s")
            nc.sync.dma_start(out=gidx_s[:], in_=gidx_d[:, :])
            scl_s = const_p.tile([128, n_sub], dt.float32, name="scl_s")
            nc.sync.dma_start(out=scl_s[:], in_=scl_d[:, :])
            w_s = []
            for i, wd_ in enumerate(w_d):
                wt = const_p.tile([128, 128], dt.float32r, tag=f"w{i}",
                                  name=f"ws{i}")
                nc.sync.dma_start(out=wt[:], in_=wd_[:, :])
                w_s.append(wt)
            blc_s = const_p.tile([128, 1], dt.float32, name="blc_s")
            nc.sync.dma_start(out=blc_s[:], in_=blc_d[:, :])
            iota_s = const_p.tile([128, 256], dt.bfloat16, name="iota_s")
            nc.sync.dma_start(out=iota_s[:], in_=iota_d[:, :])

            f32r = dt.float32r
            relu = mybir.ActivationFunctionType.Relu
            iseq = mybir.AluOpType.is_equal
            mult = mybir.AluOpType.mult

            for rep in range(reps):
                call_ctr = 0
                cur_ind = [None]
                for g in range(NG):
                    gsz = GROUP_SIZES[g]
                    gb = int(grp_base[g])
                    # inv tables for this group (overlaps with gathers)
                    invb_s = []
                    for t in range(n_types):
                        iv = invb_p.tile([128, gsz * 128], dt.float32,
                                         tag=f"invb{t}", name=f"invb{t}")
                        nc.sync.dma_start(
                            out=iv[:],
                            in_=invb_d[t][:, gb * 128:(gb + gsz) * 128])
                        invb_s.append(iv)
                    ps = {}  # (t, quad) -> [128, 512] psum bank tile
                    g_tile = None
                    cur_call = -1
                    for w in range(NW):
                        c0 = int(chunk_base[g, w, 0])
                        c1 = int(chunk_base[g, w, n_types - 1]
                                 + caps[g, w, n_types - 1])
                        for ci in range(c0, c1):
                            k = call_of_chunk[ci]
                            if k != cur_call:
                                cur_call = k
                                wn, col0, wd = calls[k]
                                b0 = wn * WBASE
                                b1 = min(b0 + WBASE, n_nodes)
                                qn = call_ctr % NQ
                                call_ctr += 1
                                g_tile = gpool.tile(
                                    [128, KG, 128], dt.bfloat16,
                                    tag=f"g{qn}", name="g")
                                nc.gpsimd.dma_gather(
                                    g_tile[:, :wd, :], hpk[b0:b1, :],
                                    gidx_s[:, col0 * 8:(col0 + wd) * 8],
                                    128 * wd, 128 * wd, 128,
                                    single_packet=False, queue_num=qn)
                            jj = ci - calls[k][1]
                            gg, ww, t = rt["chunk_cell"][ci]
                            if mode == "gather":
                                continue
                            for (sl, ns) in subs_of_chunk[ci]:
                                j = sub_id[(ci, sl)]
                                wide = ns * 128
                                if mode != "noind" or cur_ind[0] is None \
                                        or j % 8 == 0:
                                    wr = 256 if mode == "noind" else wide
                                    ind = ind_p.tile(
                                        [128, 256], dt.bfloat16,
                                        tag="ind", name="ind")
                                    cur_ind[0] = ind
                                    nc.vector.tensor_scalar(
                                        out=ind[:, :wr],
                                        in0=iota_s[:, :wr],
                                        scalar1=scl_s[:, j:j + 1],
                                        scalar2=None, op0=iseq)
                                ind = cur_ind[0]
                                if mode == "nomm":
                                    continue
                                qd = sl // 4
                                if (t, qd) not in ps:
                                    ps[(t, qd)] = psT_p.tile(
                                        [128, 512], dt.float32,
                                        tag=f"ps{t}_{qd}", name=f"ps{t}_{qd}")
                                co = (sl % 4) * 128
                                st = first_q[(t, g, qd)] == j
                                sp = last_q[(t, g, qd)] == j
                                nc.tensor.matmul(
                                    out=ps[(t, qd)][:, co:co + wide],
                                    lhsT=g_tile[:, jj, :],
                                    rhs=ind[:, :wide], start=st, stop=sp)

                    # ---------------- finalize group: weight matmuls + out
                    if mode in ("gather", "nomm"):
                        continue
                    for pl in range(gsz // 2):
                        q2 = (gb + 2 * pl) // 2
                        mts = []
                        for t in range(n_types):
                            mt = mt_p.tile([128, 256], f32r, tag=f"mt{t}",
                                           name=f"mt{t}")
                            qd, co = pl // 2, (pl % 2) * 256
                            nc.vector.tensor_tensor(
                                out=mt[:], in0=ps[(t, qd)][:, co:co + 256],
                                in1=invb_s[t][:, pl * 256:(pl + 1) * 256],
                                op=mult)
                            mts.append(mt)
                        hot_t = hot_p.tile([128, 256], f32r, tag="hot",
                                           name="hot_t")
                        nc.sync.dma_start(
                            out=hot_t[:],
                            in_=hot_d[:, q2 * 256:(q2 + 1) * 256])
                        pso = pso_p.tile([128, 256], dt.float32, tag="pso",
                                         name="pso")
                        nc.tensor.matmul(out=pso[:], lhsT=w_s[0][:],
                                         rhs=mts[0][:], start=True,
                                         stop=False)
                        nc.tensor.matmul(out=pso[:], lhsT=w_s[1][:],
                                         rhs=mts[1][:], start=False,
                                         stop=False)
                        nc.tensor.matmul(out=pso[:], lhsT=w_s[2][:],
                                         rhs=hot_t[:], start=False,
                                         stop=True)
                        ot = o_p.tile([128, 256], dt.float32, tag="ot",
                                      name="ot")
                        nc.scalar.activation(out=ot[:], in_=pso[:],
                                             func=relu, bias=blc_s[:, 0:1])
                        nc.sync.dma_start(
                            out=outT_d[:, q2 * 256:(q2 + 1) * 256],
                            in_=ot[:])

    nc.compile()
    return nc


# ------------------------------------------------------------------ driver ---

def _prepare(h, src1, dst1, src2, dst2, W1, W2, Wl, bl,
             rows_per_core=ROWS_PER_CORE, n_cores=N_CORES):
    h = np.asarray(h, np.float32)
    bl = np.asarray(bl, np.float32)
    srcs = [np.asarray(src1), np.asarray(src2)]
    dsts = [np.asarray(dst1), np.asarray(dst2)]
    rt = _route(srcs, dsts)

    hpk = np.ascontiguousarray(h.astype(BF16))  # [N, 128] bf16

    w1t = (0.5 * np.asarray(W1, np.float32).T).copy()
    w2t = (0.5 * np.asarray(W2, np.float32).T).copy()
    wlt = np.asarray(Wl, np.float32).T.copy()
    blc = bl.reshape(128, 1).copy()
    iota = np.broadcast_to(np.arange(256, dtype=np.float32), (128, 256))
    iota = np.ascontiguousarray(iota.astype(BF16))

    in_maps = []
    for c in range(n_cores):
        pc = rt["per_core"][c]
        rows = h[c * rows_per_core:(c + 1) * rows_per_core]
        pad = S * 128 - rows.shape[0]
        rows = np.pad(rows, ((0, pad), (0, 0)))
        hot = np.ascontiguousarray(rows.T)  # [128, S*128]
        in_maps.append(dict(
            hpk=hpk, gidx=pc["gidx"], scl=pc["scl"],
            invb0=pc["invb"][0], invb1=pc["invb"][1], hot=hot,
            w1t=w1t, w2t=w2t, wlt=wlt, blc=blc, iota=iota,
        ))
    return rt, in_maps


def _postprocess(results, rt, rows_per_core=ROWS_PER_CORE, n_cores=N_CORES):
    n_nodes = rows_per_core * n_cores
    out = np.empty((n_nodes, HIDDEN), np.float32)
    for c in range(n_cores):
        outT = results[c]["outT"]  # [128, S*128]
        out[c * rows_per_core:(c + 1) * rows_per_core] = \
            outT[:, :rows_per_core].T
    return out


def kernel(h, src1, dst1, src2, dst2, W1, W2, Wl, bl, **kw):
    from concourse import bass_utils
    rt, in_maps = _prepare(h, src1, dst1, src2, dst2, W1, W2, Wl, bl)
    nc = _build_program(rt, N_NODES, N_CORES)
    res = bass_utils.run_bass_kernel_spmd(
        nc, in_maps, core_ids=list(range(N_CORES)))
    return _postprocess(res.results, rt)


# revision 31
# speedup vs baseline: 1.9534x; 1.9534x over previous
"""GCN layer (2 edge types, mean aggregation + self-loop) on 8 Trainium2 cores.

Math (per reference):
    m_t = segment_mean(h[src_t] @ Wt.T, dst_t)   for t in {1,2}
    out = relu(h @ Wl.T + bl + 0.5*(m1 + m2))

Linear commutes with gather+mean: raw h rows are segment-mean'd first and
the 128x128 weights applied afterwards.

Design (v3) — measured bottlenecks drive everything:
  * dma_gather costs ~8ns/descriptor (row) regardless of row size, and
    4 SWDGE queues run near-parallel -> minimize descriptors, spread
    calls round-robin over 4 queues, gather single-bf16 rows (256B).
  * dst nodes partitioned contiguously across 8 cores (12500 = 98 slots
    of 128).  Slots processed in 9 PSUM-resident groups (8x12 + 2);
    each (type, quad-of-4-slots) owns one PSUM bank ([128, 512] f32)
    alive across all 4 src windows -> no SBUF accumulator traffic.
    Only the bank's globally-first matmul sets start (a start clears
    accumulate-bits bank-wide), only its last sets stop.
  * Segment-sum as a flipped indicator matmul
        psT[f, d] += sum_e g[e, f] * ind[e, d]
    (matmul(lhsT=g_chunk, rhs=ind)) giving the transposed mean directly.
    ind = is_equal(iota, drel) on DVE (unfused: a fused second ALU op
    measures 2x slower).  Chunks spanning two adjacent slots of a quad
    use one 256-wide indicator instead of two 128-wide ones.
  * The 1/deg mean scale is applied on the PSUM->SBUF move: one DVE
    tensor_tensor mult per (type, slot-pair) against a DMA-loaded
    partition-replicated inv table.
  * Edges routed to (core, type, group, window) cells, packed densely
    into chunks of 128 (93% fill); src indices int16 relative to one of
    4 windows of 25000 rows.
  * Final per slot-pair: 3 float32r matmuls (256-wide, full PE rate):
        out.T = relu(0.5*W1.T' m1T + 0.5*W2.T' m2T + Wl.T' hT + bl)

All 8 cores share one SPMD instruction stream; the chunk/subchunk
schedule is the max-shape over cores, per-core tables (gather indices,
drel scalar columns, inv tables) specialize it.  Padding slots gather
window row 0 and carry a sentinel drel -> indicator 0.
"""

import numpy as np
import ml_dtypes

BF16 = np.dtype(ml_dtypes.bfloat16)

# ---------------------------------------------------------------- config ---

N_NODES = 100000
HIDDEN = 128
N_CORES = 8
ROWS_PER_CORE = N_NODES // N_CORES  # 12500
S = 98                        # dst slots per core (12544 >= 12500)
GROUP_SIZES = [12] * 8 + [2]  # PSUM-resident slot groups
NW = 4                        # src windows
WBASE = 25000                 # window w covers rows [w*WBASE, (w+1)*WBASE)
KG = 4                        # chunks per dma_gather call
NQ = 4                        # SWDGE queues
SCRATCH = 32768               # dynamic DMA descriptor carveout
SENT = 1500.0               # drel sentinel (outside iota range) -> ind 0


def _cdiv(a, b):
    return -(-a // b)


# ------------------------------------------------------------ host routing ---

def _route(srcs, dsts):
    """Build the shared (static) chunk/subchunk schedule + per-core tables."""
    NG = len(GROUP_SIZES)
    grp_base = np.concatenate([[0], np.cumsum(GROUP_SIZES)[:-1]])
    grp_of = np.repeat(np.arange(NG), GROUP_SIZES)  # slot -> group

    n_types = len(srcs)
    invdeg = []
    for t in range(n_types):
        deg = np.bincount(dsts[t].astype(np.int64), minlength=N_NODES)
        invdeg.append((1.0 / np.maximum(deg, 1)).astype(np.float32))

    ed = []
    for t in range(n_types):
        src = srcs[t].astype(np.int64)
        dst = dsts[t].astype(np.int64)
        c = dst // ROWS_PER_CORE
        dl = dst - c * ROWS_PER_CORE
        s = dl >> 7
        d128 = (dl & 127).astype(np.float32)
        g = grp_of[s]
        w = src // WBASE
        idx16 = (src - w * WBASE).astype(np.int16)
        order = np.lexsort((src, s, w, g, c))
        ed.append(dict(c=c[order], s=s[order], d128=d128[order],
                       g=g[order], w=w[order], idx16=idx16[order]))

    gmax = max(GROUP_SIZES)
    cnt = np.zeros((n_types, N_CORES, NG, NW), np.int64)
    cnt_s = np.zeros((n_types, N_CORES, NG, NW, gmax), np.int64)
    for t in range(n_types):
        e = ed[t]
        s_loc = e["s"] - grp_base[e["g"]]
        np.add.at(cnt[t], (e["c"], e["g"], e["w"]), 1)
        np.add.at(cnt_s[t], (e["c"], e["g"], e["w"], s_loc), 1)

    caps = np.zeros((NG, NW, n_types), np.int64)
    for g in range(NG):
        for w in range(NW):
            for t in range(n_types):
                caps[g, w, t] = _cdiv(int(cnt[t][:, g, w].max()), 128)
    for g in range(NG):
        for t in range(n_types):
            if caps[g, :, t].sum() == 0:
                caps[g, 0, t] = 1

    chunk_base = np.zeros((NG, NW, n_types), np.int64)
    pos = 0
    for g in range(NG):
        for w in range(NW):
            for t in range(n_types):
                chunk_base[g, w, t] = pos
                pos += int(caps[g, w, t])
    n_chunks = pos

    calls = []  # (window, col0, width)
    for g in range(NG):
        for w in range(NW):
            c0 = int(chunk_base[g, w, 0])
            c1 = int(chunk_base[g, w, n_types - 1] + caps[g, w, n_types - 1])
            c = c0
            while c < c1:
                wd = min(KG, c1 - c)
                calls.append((w, c, wd))
                c += wd

    # per-chunk union (over cores) of spanned local slots
    slots_of_chunk = [set() for _ in range(n_chunks)]
    for g in range(NG):
        gsz = GROUP_SIZES[g]
        for w in range(NW):
            for t in range(n_types):
                Q = int(caps[g, w, t])
                if Q == 0:
                    continue
                base = int(chunk_base[g, w, t])
                for c in range(N_CORES):
                    cum = 0
                    for sl in range(gsz):
                        n = int(cnt_s[t][c, g, w, sl])
                        if n == 0:
                            continue
                        q0, q1 = cum // 128, (cum + n - 1) // 128
                        for q in range(q0, q1 + 1):
                            slots_of_chunk[base + q].add(sl)
                        cum += n

    # coverage injection for (t, s) with no edges anywhere
    covered = np.zeros((n_types, S), bool)
    for g in range(NG):
        for w in range(NW):
            for t in range(n_types):
                base = int(chunk_base[g, w, t])
                for q in range(int(caps[g, w, t])):
                    for sl in slots_of_chunk[base + q]:
                        covered[t, grp_base[g] + sl] = True
    for t in range(n_types):
        for s in range(S):
            if not covered[t, s]:
                g = int(grp_of[s])
                for w in range(NW):
                    if caps[g, w, t] > 0:
                        base = int(chunk_base[g, w, t])
                        slots_of_chunk[base].add(s - int(grp_base[g]))
                        break

    # merge adjacent slots (within a quad) into 256-wide subchunks
    # subs_of_chunk[ci] = [(sl_lo, n_slots)]; cover[(ci, sl)] = (j, sl_lo)
    subs_of_chunk = [[] for _ in range(n_chunks)]
    for ci in range(n_chunks):
        SL = sorted(slots_of_chunk[ci])
        i = 0
        while i < len(SL):
            sl = SL[i]
            if (i + 1 < len(SL) and SL[i + 1] == sl + 1 and sl % 4 < 3):
                subs_of_chunk[ci].append((sl, 2))
                i += 2
            else:
                subs_of_chunk[ci].append((sl, 1))
                i += 1

    # enumerate subs in stream order; pack their 1-2 indicator blocks into
    # batches of NB_BLK blocks (one DVE tensor_tensor per batch); a
    # 2-block sub never straddles a batch boundary
    NB_BLK = 8
    sub_id = {}     # (ci, sl_lo) -> j
    cover = {}      # (ci, sl) -> (j, sl_lo)
    sub_blk = {}    # j -> (batch, off)
    sub_ns = {}     # j -> blocks (1 or 2)
    chunk_cell = [None] * n_chunks
    first_q, last_q = {}, {}   # (t, g, quad) -> j
    j = 0
    batch, fill = 0, 0
    for g in range(NG):
        for w in range(NW):
            for t in range(n_types):
                base = int(chunk_base[g, w, t])
                for q in range(int(caps[g, w, t])):
                    ci = base + q
                    chunk_cell[ci] = (g, w, t)
                    for (sl, ns) in subs_of_chunk[ci]:
                        if fill + ns > NB_BLK:
                            batch += 1
                            fill = 0
                        sub_id[(ci, sl)] = j
                        sub_blk[j] = (batch, fill)
                        sub_ns[j] = ns
                        fill += ns
                        if fill == NB_BLK:
                            batch += 1
                            fill = 0
                        for k in range(ns):
                            cover[(ci, sl + k)] = (j, sl)
                        qkey = (t, g, sl // 4)
                        if qkey not in first_q:
                            first_q[qkey] = j
                        last_q[qkey] = j
                        j += 1
    n_sub = j
    n_batches = batch + (1 if fill else 0)

    # ------------------------------------------------------ per-core tables
    per_core = []
    for c in range(N_CORES):
        flat_idx = np.zeros(n_chunks * 128, np.int16)
        scl = np.full((128, n_batches * NB_BLK), SENT, np.float16)
        for t in range(n_types):
            e = ed[t]
            mask = e["c"] == c
            idx = np.nonzero(mask)[0]
            if len(idx) == 0:
                continue
            gs, ws = e["g"][idx], e["w"][idx]
            sl = e["s"][idx] - grp_base[gs]
            cellkey = gs * NW + ws
            bounds = np.nonzero(np.diff(cellkey))[0] + 1
            starts = np.concatenate([[0], bounds])
            ends = np.concatenate([bounds, [len(idx)]])
            for lo, hi in zip(starts, ends):
                g, w = int(gs[lo]), int(ws[lo])
                base = int(chunk_base[g, w, t])
                p = np.arange(hi - lo)
                eidx = idx[lo:hi]
                flat_idx[base * 128 + p] = e["idx16"][eidx]
                slr = sl[lo:hi]
                rb = np.nonzero(np.diff(slr))[0] + 1
                rst = np.concatenate([[0], rb])
                ren = np.concatenate([rb, [hi - lo]])
                for a, b in zip(rst, ren):
                    s_loc = int(slr[a])
                    for q in range(a // 128, (b - 1) // 128 + 1):
                        pa, pb = max(a, q * 128), min(b, (q + 1) * 128)
                        jj, sl_lo = cover[(base + q, s_loc)]
                        bat, off = sub_blk[jj]
                        rows = np.arange(pa, pb) % 128
                        sel = eidx[pa:pb]
                        val = (e["d128"][sel] + 128.0 * (s_loc - sl_lo)
                               + 128.0 * off).astype(np.float16)
                        for k in range(sub_ns[jj]):
                            scl[rows, bat * NB_BLK + off + k] = val

        gidx = np.zeros((128, n_chunks * 8), np.int16)
        for (w, col0, wd) in calls:
            seg = flat_idx[col0 * 128:(col0 + wd) * 128]
            gidx[:, col0 * 8:(col0 + wd) * 8] = \
                np.tile(seg.reshape(-1, 16).T, (8, 1))

        invb = []
        for t in range(n_types):
            row = np.zeros(S * 128, np.float32)
            row[:ROWS_PER_CORE] = invdeg[t][c * ROWS_PER_CORE:
                                            (c + 1) * ROWS_PER_CORE]
            invb.append(np.ascontiguousarray(
                np.broadcast_to(row, (128, S * 128))))
        per_core.append(dict(gidx=np.ascontiguousarray(gidx), scl=scl,
                             invb=invb))

    return dict(caps=caps, chunk_base=chunk_base, n_chunks=n_chunks,
                n_sub=n_sub, calls=calls, subs_of_chunk=subs_of_chunk,
                sub_id=sub_id, sub_blk=sub_blk, sub_ns=sub_ns,
                n_batches=n_batches, nb_blk=NB_BLK,
                chunk_cell=chunk_cell, first_q=first_q, last_q=last_q,
                grp_base=grp_base, per_core=per_core)


# ------------------------------------------------------------ bass program ---

def _build_program(rt, n_nodes, n_cores, reps=1):
    import os
    import concourse.bacc as bacc
    from concourse import mybir, tile, library_config

    mode = os.environ.get("KMODE", "full")  # full | gather | noind | nomm
    NG = len(GROUP_SIZES)
    n_types = 2
    caps, chunk_base = rt["caps"], rt["chunk_base"]
    n_chunks, n_sub = rt["n_chunks"], rt["n_sub"]
    calls, subs_of_chunk = rt["calls"], rt["subs_of_chunk"]
    sub_id, sub_blk, sub_ns = rt["sub_id"], rt["sub_blk"], rt["sub_ns"]
    n_batches, NB_BLK = rt["n_batches"], rt["nb_blk"]
    first_q, last_q = rt["first_q"], rt["last_q"]
    grp_base = rt["grp_base"]

    nc = bacc.Bacc("TRN2", target_bir_lowering=False, debug=False,
                   num_devices=n_cores, dynamic_dma_scratch_size=SCRATCH,
                   num_swdge_queues=NQ)
    dt = mybir.dt

    hpk = nc.dram_tensor("hpk", [n_nodes, 128], dt.float16,
                         kind="ExternalInput").ap()
    gidx_d = nc.dram_tensor("gidx", [128, n_chunks * 8], dt.int16,
                            kind="ExternalInput").ap()
    scl_d = nc.dram_tensor("scl", [128, n_batches * NB_BLK], dt.float16,
                           kind="ExternalInput").ap()
    invb_d = [nc.dram_tensor(f"invb{t}", [128, S * 128], dt.float32,
                             kind="ExternalInput").ap()
              for t in range(n_types)]
    hot_d = nc.dram_tensor("hot", [128, S * 128], dt.float32r,
                           kind="ExternalInput").ap()
    w_d = [nc.dram_tensor(w, [128, 128], dt.float32r,
                          kind="ExternalInput").ap()
           for w in ("w1t", "w2t", "wlt")]
    blc_d = nc.dram_tensor("blc", [128, 1], dt.float32,
                           kind="ExternalInput").ap()
    iota_d = nc.dram_tensor("iota", [128, NB_BLK * 128], dt.float16,
                            kind="ExternalInput").ap()
    outT_d = nc.dram_tensor("outT", [128, S * 128], dt.float32,
                            kind="ExternalOutput").ap()

    call_of_chunk = {}
    for k, (w, col0, wd) in enumerate(calls):
        for ci in range(col0, col0 + wd):
            call_of_chunk[ci] = k

    with tile.TileContext(nc) as tc:
        with (
            tc.tile_pool(name="const", bufs=1) as const_p,
            tc.tile_pool(name="gpool", bufs=6) as gpool,
            tc.tile_pool(name="ind", bufs=8) as ind_p,
            tc.tile_pool(name="mt", bufs=2) as mt_p,
            tc.tile_pool(name="invb", bufs=2) as invb_p,
            tc.tile_pool(name="hot", bufs=2) as hot_p,
            tc.tile_pool(name="ostage", bufs=2) as o_p,
            tc.tile_pool(name="psT", bufs=1, space="PSUM") as psT_p,
            tc.tile_pool(name="pso", bufs=2, space="PSUM") as pso_p,
        ):
            nc.gpsimd.load_library(library_config.mlp)
            gidx_s = const_p.tile([128, n_chunks * 8], dt.int16, name="gidx_s")
            nc.sync.dma_start(out=gidx_s[:], in_=gidx_d[:, :])
            scl_s = const_p.tile([128, n_batches * NB_BLK], dt.float16,
                                 name="scl_s")
            nc.sync.dma_start(out=scl_s[:], in_=scl_d[:, :])
            w_s = []
            for i, wd_ in enumerate(w_d):
                wt = const_p.tile([128, 128], dt.float32r, tag=f"w{i}",
                                  name=f"ws{i}")
                nc.sync.dma_start(out=wt[:], in_=wd_[:, :])
                w_s.append(wt)
            blc_s = const_p.tile([128, 1], dt.float32, name="blc_s")
            nc.sync.dma_start(out=blc_s[:], in_=blc_d[:, :])
            iota_s = const_p.tile([128, NB_BLK * 128], dt.float16,
                                  name="iota_s")
            nc.sync.dma_start(out=iota_s[:], in_=iota_d[:, :])

            f32r = dt.float32r
            relu = mybir.ActivationFunctionType.Relu
            iseq = mybir.AluOpType.is_equal
            mult = mybir.AluOpType.mult

            for rep in range(reps):
                call_ctr = 0
                ind_tiles = {}  # batch -> tile (current rep, rolling)
                for g in range(NG):
                    gsz = GROUP_SIZES[g]
                    gb = int(grp_base[g])
                    # inv tables for this group (overlaps with gathers)
                    invb_s = []
                    for t in range(n_types):
                        iv = invb_p.tile([128, gsz * 128], dt.float32,
                                         tag=f"invb{t}", name=f"invb{t}")
                        nc.sync.dma_start(
                            out=iv[:],
                            in_=invb_d[t][:, gb * 128:(gb + gsz) * 128])
                        invb_s.append(iv)
                    ps = {}  # (t, quad) -> [128, 512] psum bank tile
                    g_tile = None
                    cur_call = -1
                    for w in range(NW):
                        c0 = int(chunk_base[g, w, 0])
                        c1 = int(chunk_base[g, w, n_types - 1]
                                 + caps[g, w, n_types - 1])
                        for ci in range(c0, c1):
                            k = call_of_chunk[ci]
                            if k != cur_call:
                                cur_call = k
                                wn, col0, wd = calls[k]
                                b0 = wn * WBASE
                                b1 = min(b0 + WBASE, n_nodes)
                                qn = call_ctr % NQ
                                call_ctr += 1
                                g_tile = gpool.tile(
                                    [128, KG, 128], dt.float16,
                                    tag=f"g{qn}", name="g")
                                nc.gpsimd.dma_gather(
                                    g_tile[:, :wd, :], hpk[b0:b1, :],
                                    gidx_s[:, col0 * 8:(col0 + wd) * 8],
                                    128 * wd, 128 * wd, 128,
                                    single_packet=False, queue_num=qn)
                            jj = ci - calls[k][1]
                            gg, ww, t = rt["chunk_cell"][ci]
                            if mode == "gather":
                                continue
                            for (sl, ns) in subs_of_chunk[ci]:
                                j = sub_id[(ci, sl)]
                                wide = ns * 128
                                bat, off = sub_blk[j]
                                if bat not in ind_tiles and (
                                        mode != "noind" or not ind_tiles
                                        or bat % 8 == 0):
                                    ind = ind_p.tile(
                                        [128, NB_BLK * 128], dt.float16,
                                        tag="ind", name="ind")
                                    from concourse import bass as _bass
                                    slc = scl_s[:, bat * NB_BLK:
                                                (bat + 1) * NB_BLK]
                                    bc = _bass.AP(
                                        slc.tensor, slc.offset,
                                        slc.ap + [[0, 128]])
                                    nc.vector.tensor_tensor(
                                        out=ind[:], in0=iota_s[:],
                                        in1=bc, op=iseq)
                                    ind_tiles = {bat: ind}
                                elif bat not in ind_tiles:
                                    ind_tiles = {bat: ind}
                                ind = ind_tiles[bat]
                                if mode == "nomm":
                                    continue
                                qd = sl // 4
                                if (t, qd) not in ps:
                                    ps[(t, qd)] = psT_p.tile(
                                        [128, 512], dt.float32,
                                        tag=f"ps{t}_{qd}", name=f"ps{t}_{qd}")
                                co = (sl % 4) * 128
                                st = first_q[(t, g, qd)] == j
                                sp = last_q[(t, g, qd)] == j
                                nc.tensor.matmul(
                                    out=ps[(t, qd)][:, co:co + wide],
                                    lhsT=g_tile[:, jj, :],
                                    rhs=ind[:, off * 128:off * 128 + wide],
                                    start=st, stop=sp)

                    # ---------------- finalize group: weight matmuls + out
                    if mode in ("gather", "nomm"):
                        continue
                    for pl in range(gsz // 2):
                        q2 = (gb + 2 * pl) // 2
                        mts = []
                        for t in range(n_types):
                            mt = mt_p.tile([128, 256], f32r, tag=f"mt{t}",
                                           name=f"mt{t}")
                            qd, co = pl // 2, (pl % 2) * 256
                            nc.vector.tensor_tensor(
                                out=mt[:], in0=ps[(t, qd)][:, co:co + 256],
                                in1=invb_s[t][:, pl * 256:(pl + 1) * 256],
                                op=mult)
                            mts.append(mt)
                        hot_t = hot_p.tile([128, 256], f32r, tag="hot",
                                           name="hot_t")
                        nc.sync.dma_start(
                            out=hot_t[:],
                            in_=hot_d[:, q2 * 256:(q2 + 1) * 256])
                        pso = pso_p.tile([128, 256], dt.float32, tag="pso",
                                         name="pso")
                        nc.tensor.matmul(out=pso[:], lhsT=w_s[0][:],
                                         rhs=mts[0][:], start=True,
                                         stop=False)
                        nc.tensor.matmul(out=pso[:], lhsT=w_s[1][:],
                                         rhs=mts[1][:], start=False,
                                         stop=False)
                        nc.tensor.matmul(out=pso[:], lhsT=w_s[2][:],
                                         rhs=hot_t[:], start=False,
                                         stop=True)
                        ot = o_p.tile([128, 256], dt.float32, tag="ot",
                                      name="ot")
                        nc.scalar.activation(out=ot[:], in_=pso[:],
                                             func=relu, bias=blc_s[:, 0:1])
                        nc.sync.dma_start(
                            out=outT_d[:, q2 * 256:(q2 + 1) * 256],
                            in_=ot[:])

    nc.compile()
    return nc


# ------------------------------------------------------------------ driver ---

def _prepare(h, src1, dst1, src2, dst2, W1, W2, Wl, bl,
             rows_per_core=ROWS_PER_CORE, n_cores=N_CORES):
    h = np.asarray(h, np.float32)
    bl = np.asarray(bl, np.float32)
    srcs = [np.asarray(src1), np.asarray(src2)]
    dsts = [np.asarray(dst1), np.asarray(dst2)]
    rt = _route(srcs, dsts)

    hpk = np.ascontiguousarray(h.astype(np.float16))  # [N, 128] fp16

    w1t = (0.5 * np.asarray(W1, np.float32).T).copy()
    w2t = (0.5 * np.asarray(W2, np.float32).T).copy()
    wlt = np.asarray(Wl, np.float32).T.copy()
    blc = bl.reshape(128, 1).copy()
    W_IND = rt["nb_blk"] * 128
    iota = np.broadcast_to(np.arange(W_IND, dtype=np.float32),
                           (128, W_IND))
    iota = np.ascontiguousarray(iota.astype(np.float16))

    in_maps = []
    for c in range(n_cores):
        pc = rt["per_core"][c]
        rows = h[c * rows_per_core:(c + 1) * rows_per_core]
        pad = S * 128 - rows.shape[0]
        rows = np.pad(rows, ((0, pad), (0, 0)))
        hot = np.ascontiguousarray(rows.T)  # [128, S*128]
        in_maps.append(dict(
            hpk=hpk, gidx=pc["gidx"], scl=pc["scl"],
            invb0=pc["invb"][0], invb1=pc["invb"][1], hot=hot,
            w1t=w1t, w2t=w2t, wlt=wlt, blc=blc, iota=iota,
        ))
    return rt, in_maps


def _postprocess(results, rt, rows_per_core=ROWS_PER_CORE, n_cores=N_CORES):
    n_nodes = rows_per_core * n_cores
    out = np.empty((n_nodes, HIDDEN), np.float32)
    for c in range(n_cores):
        outT = results[c]["outT"]  # [128, S*128]
        out[c * rows_per_core:(c + 1) * rows_per_core] = \
            outT[:, :rows_per_core].T
    return out


def kernel(h, src1, dst1, src2, dst2, W1, W2, Wl, bl, **kw):
    from concourse import bass_utils
    rt, in_maps = _prepare(h, src1, dst1, src2, dst2, W1, W2, Wl, bl)
    nc = _build_program(rt, N_NODES, N_CORES)
    res = bass_utils.run_bass_kernel_spmd(
        nc, in_maps, core_ids=list(range(N_CORES)))
    return _postprocess(res.results, rt)


# revision 33
# speedup vs baseline: 2.1401x; 1.0956x over previous
"""GCN layer (2 edge types, mean aggregation + self-loop) on 8 Trainium2 cores.

Math (per reference):
    m_t = segment_mean(h[src_t] @ Wt.T, dst_t)   for t in {1,2}
    out = relu(h @ Wl.T + bl + 0.5*(m1 + m2))

Linear commutes with gather+mean: raw h rows are segment-mean'd first and
the 128x128 weights applied afterwards.

Design (v3) — measured bottlenecks drive everything:
  * dma_gather costs ~8ns/descriptor (row) regardless of row size, and
    4 SWDGE queues run near-parallel -> minimize descriptors, spread
    calls round-robin over 4 queues, gather single-bf16 rows (256B).
  * dst nodes partitioned contiguously across 8 cores (12500 = 98 slots
    of 128).  Slots processed in 9 PSUM-resident groups (8x12 + 2);
    each (type, quad-of-4-slots) owns one PSUM bank ([128, 512] f32)
    alive across all 4 src windows -> no SBUF accumulator traffic.
    Only the bank's globally-first matmul sets start (a start clears
    accumulate-bits bank-wide), only its last sets stop.
  * Segment-sum as a flipped indicator matmul
        psT[f, d] += sum_e g[e, f] * ind[e, d]
    (matmul(lhsT=g_chunk, rhs=ind)) giving the transposed mean directly.
    ind = is_equal(iota, drel) on DVE (unfused: a fused second ALU op
    measures 2x slower).  Chunks spanning two adjacent slots of a quad
    use one 256-wide indicator instead of two 128-wide ones.
  * The 1/deg mean scale is applied on the PSUM->SBUF move: one DVE
    tensor_tensor mult per (type, slot-pair) against a DMA-loaded
    partition-replicated inv table.
  * Edges routed to (core, type, group, window) cells, packed densely
    into chunks of 128 (93% fill); src indices int16 relative to one of
    4 windows of 25000 rows.
  * Final per slot-pair: 3 float32r matmuls (256-wide, full PE rate):
        out.T = relu(0.5*W1.T' m1T + 0.5*W2.T' m2T + Wl.T' hT + bl)

All 8 cores share one SPMD instruction stream; the chunk/subchunk
schedule is the max-shape over cores, per-core tables (gather indices,
drel scalar columns, inv tables) specialize it.  Padding slots gather
window row 0 and carry a sentinel drel -> indicator 0.
"""

import numpy as np
import ml_dtypes

BF16 = np.dtype(ml_dtypes.bfloat16)

# ---------------------------------------------------------------- config ---

import os as _os

N_NODES = 100000
HIDDEN = 128
N_CORES = 8
ROWS_PER_CORE = N_NODES // N_CORES  # 12500
S = 98                        # dst slots per core (12544 >= 12500)
GROUP_SIZES = [12] * 8 + [2]  # PSUM-resident slot groups
NW = 4                        # src windows
WBASE = 25000                 # window w covers rows [w*WBASE, (w+1)*WBASE)
KG = int(_os.environ.get("KKG", "4"))   # chunks per dma_gather call
NQ = 4                        # SWDGE queues
SCRATCH = int(_os.environ.get("KSCRATCH", "32768"))  # descriptor carveout
SP = _os.environ.get("KSP", "0") == "1"  # dma_gather single_packet
SENT = 1500.0               # drel sentinel (outside iota range) -> ind 0


def _cdiv(a, b):
    return -(-a // b)


# ------------------------------------------------------------ host routing ---

def _route(srcs, dsts):
    """Build the shared (static) chunk/subchunk schedule + per-core tables."""
    NG = len(GROUP_SIZES)
    grp_base = np.concatenate([[0], np.cumsum(GROUP_SIZES)[:-1]])
    grp_of = np.repeat(np.arange(NG), GROUP_SIZES)  # slot -> group

    n_types = len(srcs)
    invdeg = []
    for t in range(n_types):
        deg = np.bincount(dsts[t].astype(np.int64), minlength=N_NODES)
        invdeg.append((1.0 / np.maximum(deg, 1)).astype(np.float32))

    ed = []
    for t in range(n_types):
        src = srcs[t].astype(np.int64)
        dst = dsts[t].astype(np.int64)
        c = dst // ROWS_PER_CORE
        dl = dst - c * ROWS_PER_CORE
        s = dl >> 7
        d128 = (dl & 127).astype(np.float32)
        g = grp_of[s]
        w = src // WBASE
        idx16 = (src - w * WBASE).astype(np.int16)
        order = np.lexsort((src, s, w, g, c))
        ed.append(dict(c=c[order], s=s[order], d128=d128[order],
                       g=g[order], w=w[order], idx16=idx16[order]))

    gmax = max(GROUP_SIZES)
    cnt = np.zeros((n_types, N_CORES, NG, NW), np.int64)
    cnt_s = np.zeros((n_types, N_CORES, NG, NW, gmax), np.int64)
    for t in range(n_types):
        e = ed[t]
        s_loc = e["s"] - grp_base[e["g"]]
        np.add.at(cnt[t], (e["c"], e["g"], e["w"]), 1)
        np.add.at(cnt_s[t], (e["c"], e["g"], e["w"], s_loc), 1)

    caps = np.zeros((NG, NW, n_types), np.int64)
    for g in range(NG):
        for w in range(NW):
            for t in range(n_types):
                caps[g, w, t] = _cdiv(int(cnt[t][:, g, w].max()), 128)
    for g in range(NG):
        for t in range(n_types):
            if caps[g, :, t].sum() == 0:
                caps[g, 0, t] = 1

    chunk_base = np.zeros((NG, NW, n_types), np.int64)
    pos = 0
    for g in range(NG):
        for w in range(NW):
            for t in range(n_types):
                chunk_base[g, w, t] = pos
                pos += int(caps[g, w, t])
    n_chunks = pos

    calls = []  # (window, col0, width)
    for g in range(NG):
        for w in range(NW):
            c0 = int(chunk_base[g, w, 0])
            c1 = int(chunk_base[g, w, n_types - 1] + caps[g, w, n_types - 1])
            c = c0
            while c < c1:
                wd = min(KG, c1 - c)
                calls.append((w, c, wd))
                c += wd

    # per-chunk union (over cores) of spanned local slots
    slots_of_chunk = [set() for _ in range(n_chunks)]
    for g in range(NG):
        gsz = GROUP_SIZES[g]
        for w in range(NW):
            for t in range(n_types):
                Q = int(caps[g, w, t])
                if Q == 0:
                    continue
                base = int(chunk_base[g, w, t])
                for c in range(N_CORES):
                    cum = 0
                    for sl in range(gsz):
                        n = int(cnt_s[t][c, g, w, sl])
                        if n == 0:
                            continue
                        q0, q1 = cum // 128, (cum + n - 1) // 128
                        for q in range(q0, q1 + 1):
                            slots_of_chunk[base + q].add(sl)
                        cum += n

    # coverage injection for (t, s) with no edges anywhere
    covered = np.zeros((n_types, S), bool)
    for g in range(NG):
        for w in range(NW):
            for t in range(n_types):
                base = int(chunk_base[g, w, t])
                for q in range(int(caps[g, w, t])):
                    for sl in slots_of_chunk[base + q]:
                        covered[t, grp_base[g] + sl] = True
    for t in range(n_types):
        for s in range(S):
            if not covered[t, s]:
                g = int(grp_of[s])
                for w in range(NW):
                    if caps[g, w, t] > 0:
                        base = int(chunk_base[g, w, t])
                        slots_of_chunk[base].add(s - int(grp_base[g]))
                        break

    # merge adjacent slots (within a quad) into 256-wide subchunks
    # subs_of_chunk[ci] = [(sl_lo, n_slots)]; cover[(ci, sl)] = (j, sl_lo)
    subs_of_chunk = [[] for _ in range(n_chunks)]
    for ci in range(n_chunks):
        SL = sorted(slots_of_chunk[ci])
        i = 0
        while i < len(SL):
            sl = SL[i]
            if (i + 1 < len(SL) and SL[i + 1] == sl + 1 and sl % 4 < 3):
                subs_of_chunk[ci].append((sl, 2))
                i += 2
            else:
                subs_of_chunk[ci].append((sl, 1))
                i += 1

    # enumerate subs in stream order; pack their 1-2 indicator blocks into
    # batches of NB_BLK blocks (one DVE tensor_tensor per batch); a
    # 2-block sub never straddles a batch boundary
    NB_BLK = 8
    sub_id = {}     # (ci, sl_lo) -> j
    cover = {}      # (ci, sl) -> (j, sl_lo)
    sub_blk = {}    # j -> (batch, off)
    sub_ns = {}     # j -> blocks (1 or 2)
    chunk_cell = [None] * n_chunks
    first_q, last_q = {}, {}   # (t, g, quad) -> j
    j = 0
    batch, fill = 0, 0
    for g in range(NG):
        for w in range(NW):
            for t in range(n_types):
                base = int(chunk_base[g, w, t])
                for q in range(int(caps[g, w, t])):
                    ci = base + q
                    chunk_cell[ci] = (g, w, t)
                    for (sl, ns) in subs_of_chunk[ci]:
                        if fill + ns > NB_BLK:
                            batch += 1
                            fill = 0
                        sub_id[(ci, sl)] = j
                        sub_blk[j] = (batch, fill)
                        sub_ns[j] = ns
                        fill += ns
                        if fill == NB_BLK:
                            batch += 1
                            fill = 0
                        for k in range(ns):
                            cover[(ci, sl + k)] = (j, sl)
                        qkey = (t, g, sl // 4)
                        if qkey not in first_q:
                            first_q[qkey] = j
                        last_q[qkey] = j
                        j += 1
    n_sub = j
    n_batches = batch + (1 if fill else 0)

    # ------------------------------------------------------ per-core tables
    per_core = []
    for c in range(N_CORES):
        flat_idx = np.zeros(n_chunks * 128, np.int16)
        scl = np.full((128, n_batches * NB_BLK), SENT, np.float16)
        for t in range(n_types):
            e = ed[t]
            mask = e["c"] == c
            idx = np.nonzero(mask)[0]
            if len(idx) == 0:
                continue
            gs, ws = e["g"][idx], e["w"][idx]
            sl = e["s"][idx] - grp_base[gs]
            cellkey = gs * NW + ws
            bounds = np.nonzero(np.diff(cellkey))[0] + 1
            starts = np.concatenate([[0], bounds])
            ends = np.concatenate([bounds, [len(idx)]])
            for lo, hi in zip(starts, ends):
                g, w = int(gs[lo]), int(ws[lo])
                base = int(chunk_base[g, w, t])
                p = np.arange(hi - lo)
                eidx = idx[lo:hi]
                flat_idx[base * 128 + p] = e["idx16"][eidx]
                slr = sl[lo:hi]
                rb = np.nonzero(np.diff(slr))[0] + 1
                rst = np.concatenate([[0], rb])
                ren = np.concatenate([rb, [hi - lo]])
                for a, b in zip(rst, ren):
                    s_loc = int(slr[a])
                    for q in range(a // 128, (b - 1) // 128 + 1):
                        pa, pb = max(a, q * 128), min(b, (q + 1) * 128)
                        jj, sl_lo = cover[(base + q, s_loc)]
                        bat, off = sub_blk[jj]
                        rows = np.arange(pa, pb) % 128
                        sel = eidx[pa:pb]
                        val = (e["d128"][sel] + 128.0 * (s_loc - sl_lo)
                               + 128.0 * off).astype(np.float16)
                        for k in range(sub_ns[jj]):
                            scl[rows, bat * NB_BLK + off + k] = val

        gidx = np.zeros((128, n_chunks * 8), np.int16)
        for (w, col0, wd) in calls:
            seg = flat_idx[col0 * 128:(col0 + wd) * 128]
            gidx[:, col0 * 8:(col0 + wd) * 8] = \
                np.tile(seg.reshape(-1, 16).T, (8, 1))

        invb = []
        for t in range(n_types):
            row = np.zeros(S * 128, np.float32)
            row[:ROWS_PER_CORE] = invdeg[t][c * ROWS_PER_CORE:
                                            (c + 1) * ROWS_PER_CORE]
            invb.append(np.ascontiguousarray(
                np.broadcast_to(row, (128, S * 128))))
        per_core.append(dict(gidx=np.ascontiguousarray(gidx), scl=scl,
                             invb=invb))

    return dict(caps=caps, chunk_base=chunk_base, n_chunks=n_chunks,
                n_sub=n_sub, calls=calls, subs_of_chunk=subs_of_chunk,
                sub_id=sub_id, sub_blk=sub_blk, sub_ns=sub_ns,
                n_batches=n_batches, nb_blk=NB_BLK,
                chunk_cell=chunk_cell, first_q=first_q, last_q=last_q,
                grp_base=grp_base, per_core=per_core)


# ------------------------------------------------------------ bass program ---

def _build_program(rt, n_nodes, n_cores, reps=1):
    import os
    import concourse.bacc as bacc
    from concourse import mybir, tile, library_config

    mode = os.environ.get("KMODE", "full")  # full | gather | noind | nomm
    NG = len(GROUP_SIZES)
    n_types = 2
    caps, chunk_base = rt["caps"], rt["chunk_base"]
    n_chunks, n_sub = rt["n_chunks"], rt["n_sub"]
    calls, subs_of_chunk = rt["calls"], rt["subs_of_chunk"]
    sub_id, sub_blk, sub_ns = rt["sub_id"], rt["sub_blk"], rt["sub_ns"]
    n_batches, NB_BLK = rt["n_batches"], rt["nb_blk"]
    first_q, last_q = rt["first_q"], rt["last_q"]
    grp_base = rt["grp_base"]

    nc = bacc.Bacc("TRN2", target_bir_lowering=False, debug=False,
                   num_devices=n_cores, dynamic_dma_scratch_size=SCRATCH,
                   num_swdge_queues=NQ)
    dt = mybir.dt

    hpk = nc.dram_tensor("hpk", [n_nodes, 128], dt.float16,
                         kind="ExternalInput").ap()
    gidx_d = nc.dram_tensor("gidx", [128, n_chunks * 8], dt.int16,
                            kind="ExternalInput").ap()
    scl_d = nc.dram_tensor("scl", [128, n_batches * NB_BLK], dt.float16,
                           kind="ExternalInput").ap()
    invb_d = [nc.dram_tensor(f"invb{t}", [128, S * 128], dt.float32,
                             kind="ExternalInput").ap()
              for t in range(n_types)]
    hot_d = nc.dram_tensor("hot", [128, S * 128], dt.float32r,
                           kind="ExternalInput").ap()
    w_d = [nc.dram_tensor(w, [128, 128], dt.float32r,
                          kind="ExternalInput").ap()
           for w in ("w1t", "w2t", "wlt")]
    blc_d = nc.dram_tensor("blc", [128, 1], dt.float32,
                           kind="ExternalInput").ap()
    iota_d = nc.dram_tensor("iota", [128, NB_BLK * 128], dt.float16,
                            kind="ExternalInput").ap()
    outT_d = nc.dram_tensor("outT", [128, S * 128], dt.float32,
                            kind="ExternalOutput").ap()

    call_of_chunk = {}
    for k, (w, col0, wd) in enumerate(calls):
        for ci in range(col0, col0 + wd):
            call_of_chunk[ci] = k

    with tile.TileContext(nc) as tc:
        with (
            tc.tile_pool(name="const", bufs=1) as const_p,
            tc.tile_pool(name="gpool", bufs=6) as gpool,
            tc.tile_pool(name="ind", bufs=8) as ind_p,
            tc.tile_pool(name="mt", bufs=2) as mt_p,
            tc.tile_pool(name="invb", bufs=2) as invb_p,
            tc.tile_pool(name="hot", bufs=2) as hot_p,
            tc.tile_pool(name="ostage", bufs=2) as o_p,
            tc.tile_pool(name="psT", bufs=1, space="PSUM") as psT_p,
            tc.tile_pool(name="pso", bufs=2, space="PSUM") as pso_p,
        ):
            nc.gpsimd.load_library(library_config.mlp)
            gidx_s = const_p.tile([128, n_chunks * 8], dt.int16, name="gidx_s")
            nc.sync.dma_start(out=gidx_s[:], in_=gidx_d[:, :])
            scl_s = const_p.tile([128, n_batches * NB_BLK], dt.float16,
                                 name="scl_s")
            nc.sync.dma_start(out=scl_s[:], in_=scl_d[:, :])
            w_s = []
            for i, wd_ in enumerate(w_d):
                wt = const_p.tile([128, 128], dt.float32r, tag=f"w{i}",
                                  name=f"ws{i}")
                nc.sync.dma_start(out=wt[:], in_=wd_[:, :])
                w_s.append(wt)
            blc_s = const_p.tile([128, 1], dt.float32, name="blc_s")
            nc.sync.dma_start(out=blc_s[:], in_=blc_d[:, :])
            iota_s = const_p.tile([128, NB_BLK * 128], dt.float16,
                                  name="iota_s")
            nc.sync.dma_start(out=iota_s[:], in_=iota_d[:, :])

            f32r = dt.float32r
            relu = mybir.ActivationFunctionType.Relu
            iseq = mybir.AluOpType.is_equal
            mult = mybir.AluOpType.mult

            for rep in range(reps):
                call_ctr = 0
                ind_tiles = {}  # batch -> tile (current rep, rolling)
                for g in range(NG):
                    gsz = GROUP_SIZES[g]
                    gb = int(grp_base[g])
                    # inv tables for this group (overlaps with gathers)
                    invb_s = []
                    for t in range(n_types):
                        iv = invb_p.tile([128, gsz * 128], dt.float32,
                                         tag=f"invb{t}", name=f"invb{t}")
                        nc.sync.dma_start(
                            out=iv[:],
                            in_=invb_d[t][:, gb * 128:(gb + gsz) * 128])
                        invb_s.append(iv)
                    ps = {}  # (t, quad) -> [128, 512] psum bank tile
                    g_tile = None
                    cur_call = -1
                    for w in range(NW):
                        c0 = int(chunk_base[g, w, 0])
                        c1 = int(chunk_base[g, w, n_types - 1]
                                 + caps[g, w, n_types - 1])
                        for ci in range(c0, c1):
                            k = call_of_chunk[ci]
                            if k != cur_call:
                                cur_call = k
                                wn, col0, wd = calls[k]
                                b0 = wn * WBASE
                                b1 = min(b0 + WBASE, n_nodes)
                                qn = call_ctr % NQ
                                call_ctr += 1
                                g_tile = gpool.tile(
                                    [128, KG, 128], dt.float16,
                                    tag=f"g{qn}", name="g")
                                nc.gpsimd.dma_gather(
                                    g_tile[:, :wd, :], hpk[b0:b1, :],
                                    gidx_s[:, col0 * 8:(col0 + wd) * 8],
                                    128 * wd, 128 * wd, 128,
                                    single_packet=SP, queue_num=qn)
                            jj = ci - calls[k][1]
                            gg, ww, t = rt["chunk_cell"][ci]
                            if mode == "gather":
                                continue
                            for (sl, ns) in subs_of_chunk[ci]:
                                j = sub_id[(ci, sl)]
                                wide = ns * 128
                                bat, off = sub_blk[j]
                                if bat not in ind_tiles and (
                                        mode != "noind" or not ind_tiles
                                        or bat % 8 == 0):
                                    ind = ind_p.tile(
                                        [128, NB_BLK * 128], dt.float16,
                                        tag="ind", name="ind")
                                    from concourse import bass as _bass
                                    slc = scl_s[:, bat * NB_BLK:
                                                (bat + 1) * NB_BLK]
                                    bc = _bass.AP(
                                        slc.tensor, slc.offset,
                                        slc.ap + [[0, 128]])
                                    nc.vector.tensor_tensor(
                                        out=ind[:], in0=iota_s[:],
                                        in1=bc, op=iseq)
                                    ind_tiles = {bat: ind}
                                elif bat not in ind_tiles:
                                    ind_tiles = {bat: ind}
                                ind = ind_tiles[bat]
                                if mode == "nomm":
                                    continue
                                qd = sl // 4
                                if (t, qd) not in ps:
                                    ps[(t, qd)] = psT_p.tile(
                                        [128, 512], dt.float32,
                                        tag=f"ps{t}_{qd}", name=f"ps{t}_{qd}")
                                co = (sl % 4) * 128
                                st = first_q[(t, g, qd)] == j
                                sp = last_q[(t, g, qd)] == j
                                nc.tensor.matmul(
                                    out=ps[(t, qd)][:, co:co + wide],
                                    lhsT=g_tile[:, jj, :],
                                    rhs=ind[:, off * 128:off * 128 + wide],
                                    start=st, stop=sp)

                    # ---------------- finalize group: weight matmuls + out
                    if mode in ("gather", "nomm"):
                        continue
                    for pl in range(gsz // 2):
                        q2 = (gb + 2 * pl) // 2
                        mts = []
                        for t in range(n_types):
                            mt = mt_p.tile([128, 256], f32r, tag=f"mt{t}",
                                           name=f"mt{t}")
                            qd, co = pl // 2, (pl % 2) * 256
                            nc.vector.tensor_tensor(
                                out=mt[:], in0=ps[(t, qd)][:, co:co + 256],
                                in1=invb_s[t][:, pl * 256:(pl + 1) * 256],
                                op=mult)
                            mts.append(mt)
                        hot_t = hot_p.tile([128, 256], f32r, tag="hot",
                                           name="hot_t")
                        nc.sync.dma_start(
                            out=hot_t[:],
                            in_=hot_d[:, q2 * 256:(q2 + 1) * 256])
                        pso = pso_p.tile([128, 256], dt.float32, tag="pso",
                                         name="pso")
                        nc.tensor.matmul(out=pso[:], lhsT=w_s[0][:],
                                         rhs=mts[0][:], start=True,
                                         stop=False)
                        nc.tensor.matmul(out=pso[:], lhsT=w_s[1][:],
                                         rhs=mts[1][:], start=False,
                                         stop=False)
                        nc.tensor.matmul(out=pso[:], lhsT=w_s[2][:],
                                         rhs=hot_t[:], start=False,
                                         stop=True)
                        ot = o_p.tile([128, 256], dt.float32, tag="ot",
                                      name="ot")
                        nc.scalar.activation(out=ot[:], in_=pso[:],
                                             func=relu, bias=blc_s[:, 0:1])
                        nc.sync.dma_start(
                            out=outT_d[:, q2 * 256:(q2 + 1) * 256],
                            in_=ot[:])

    nc.compile()
    return nc


# ------------------------------------------------------------------ driver ---

def _prepare(h, src1, dst1, src2, dst2, W1, W2, Wl, bl,
             rows_per_core=ROWS_PER_CORE, n_cores=N_CORES):
    h = np.asarray(h, np.float32)
    bl = np.asarray(bl, np.float32)
    srcs = [np.asarray(src1), np.asarray(src2)]
    dsts = [np.asarray(dst1), np.asarray(dst2)]
    rt = _route(srcs, dsts)

    hpk = np.ascontiguousarray(h.astype(np.float16))  # [N, 128] fp16

    w1t = (0.5 * np.asarray(W1, np.float32).T).copy()
    w2t = (0.5 * np.asarray(W2, np.float32).T).copy()
    wlt = np.asarray(Wl, np.float32).T.copy()
    blc = bl.reshape(128, 1).copy()
    W_IND = rt["nb_blk"] * 128
    iota = np.broadcast_to(np.arange(W_IND, dtype=np.float32),
                           (128, W_IND))
    iota = np.ascontiguousarray(iota.astype(np.float16))

    in_maps = []
    for c in range(n_cores):
        pc = rt["per_core"][c]
        rows = h[c * rows_per_core:(c + 1) * rows_per_core]
        pad = S * 128 - rows.shape[0]
        rows = np.pad(rows, ((0, pad), (0, 0)))
        hot = np.ascontiguousarray(rows.T)  # [128, S*128]
        in_maps.append(dict(
            hpk=hpk, gidx=pc["gidx"], scl=pc["scl"],
            invb0=pc["invb"][0], invb1=pc["invb"][1], hot=hot,
            w1t=w1t, w2t=w2t, wlt=wlt, blc=blc, iota=iota,
        ))
    return rt, in_maps


def _postprocess(results, rt, rows_per_core=ROWS_PER_CORE, n_cores=N_CORES):
    n_nodes = rows_per_core * n_cores
    out = np.empty((n_nodes, HIDDEN), np.float32)
    for c in range(n_cores):
        outT = results[c]["outT"]  # [128, S*128]
        out[c * rows_per_core:(c + 1) * rows_per_core] = \
            outT[:, :rows_per_core].T
    return out


def kernel(h, src1, dst1, src2, dst2, W1, W2, Wl, bl, **kw):
    from concourse import bass_utils
    rt, in_maps = _prepare(h, src1, dst1, src2, dst2, W1, W2, Wl, bl)
    nc = _build_program(rt, N_NODES, N_CORES)
    res = bass_utils.run_bass_kernel_spmd(
        nc, in_maps, core_ids=list(range(N_CORES)))
    return _postprocess(res.results, rt)


# revision 34
# speedup vs baseline: 2.2815x; 1.0661x over previous
"""GCN layer (2 edge types, mean aggregation + self-loop) on 8 Trainium2 cores.

Math (per reference):
    m_t = segment_mean(h[src_t] @ Wt.T, dst_t)   for t in {1,2}
    out = relu(h @ Wl.T + bl + 0.5*(m1 + m2))

Linear commutes with gather+mean: raw h rows are segment-mean'd first and
the 128x128 weights applied afterwards.

Design (v4) — measured bottlenecks drive everything:
  * dma_gather costs ~2-8ns/descriptor (row) regardless of row size;
    4 SWDGE queues run near-parallel and single_packet=True is faster
    -> minimize descriptors, spread KG=4-chunk calls round-robin over 4
    queues, gather single-fp16 rows (256B).
  * dst nodes partitioned contiguously across 8 cores (12500 = 98 slots
    of 128).  Slots processed in 9 PSUM-resident groups (8x12 + 2);
    each (type, quad-of-4-slots) owns one PSUM bank ([128, 512] f32)
    alive across all 4 src windows -> no SBUF accumulator traffic.
    Only the bank's globally-first matmul sets start (a start clears
    accumulate-bits bank-wide), only its last sets stop.
  * Segment-sum as a flipped indicator matmul
        psT[f, d] += sum_e g[e, f] * ind[e, d]
    (matmul(lhsT=g_chunk, rhs=ind)) giving the transposed mean directly.
    ind = is_equal(iota, drel) on DVE (unfused: a fused second ALU op
    measures 2x slower).  Chunks spanning two adjacent slots of a quad
    use one 256-wide indicator instead of two 128-wide ones, and the
    per-instruction overhead is amortized by batching 8 indicator
    blocks into ONE tensor_tensor is_equal: in0 = a [128, 1024] fp16
    iota (0..1023), in1 = 8 scl columns broadcast along the free dim
    via a stride-0 AP; each sub's drel values are pre-biased by
    128*block_offset on the host.  fp16 keeps integers <= 2048 exact.
  * The 1/deg mean scale is applied on the PSUM->SBUF move: one DVE
    tensor_tensor mult per (type, slot-pair) against a DMA-loaded
    partition-replicated inv table.
  * Edges routed to (core, type, group, window) cells, packed densely
    into chunks of 128 (93% fill); src indices int16 relative to one of
    4 windows of 25000 rows.
  * Final per slot-pair: 3 float32r matmuls (256-wide, full PE rate):
        out.T = relu(0.5*W1.T' m1T + 0.5*W2.T' m2T + Wl.T' hT + bl)

All 8 cores share one SPMD instruction stream; the chunk/subchunk
schedule is the max-shape over cores, per-core tables (gather indices,
drel scalar columns, inv tables) specialize it.  Padding slots gather
window row 0 and carry a sentinel drel -> indicator 0.
"""

import numpy as np
import ml_dtypes

BF16 = np.dtype(ml_dtypes.bfloat16)

# ---------------------------------------------------------------- config ---

import os as _os

N_NODES = 100000
HIDDEN = 128
N_CORES = 8
ROWS_PER_CORE = N_NODES // N_CORES  # 12500
S = 98                        # dst slots per core (12544 >= 12500)
GROUP_SIZES = [12] * 8 + [2]  # PSUM-resident slot groups
NW = 4                        # src windows
WBASE = 25000                 # window w covers rows [w*WBASE, (w+1)*WBASE)
KG = int(_os.environ.get("KKG", "4"))   # chunks per dma_gather call
NQ = 4                        # SWDGE queues
SCRATCH = int(_os.environ.get("KSCRATCH", "32768"))  # descriptor carveout
SP = _os.environ.get("KSP", "1") == "1"  # dma_gather single_packet
SENT = 1500.0               # drel sentinel (outside iota range) -> ind 0


def _cdiv(a, b):
    return -(-a // b)


# ------------------------------------------------------------ host routing ---

def _route(srcs, dsts):
    """Build the shared (static) chunk/subchunk schedule + per-core tables."""
    NG = len(GROUP_SIZES)
    grp_base = np.concatenate([[0], np.cumsum(GROUP_SIZES)[:-1]])
    grp_of = np.repeat(np.arange(NG), GROUP_SIZES)  # slot -> group

    n_types = len(srcs)
    invdeg = []
    for t in range(n_types):
        deg = np.bincount(dsts[t].astype(np.int64), minlength=N_NODES)
        invdeg.append((1.0 / np.maximum(deg, 1)).astype(np.float32))

    ed = []
    for t in range(n_types):
        src = srcs[t].astype(np.int64)
        dst = dsts[t].astype(np.int64)
        c = dst // ROWS_PER_CORE
        dl = dst - c * ROWS_PER_CORE
        s = dl >> 7
        d128 = (dl & 127).astype(np.float32)
        g = grp_of[s]
        w = src // WBASE
        idx16 = (src - w * WBASE).astype(np.int16)
        order = np.lexsort((src, s, w, g, c))
        ed.append(dict(c=c[order], s=s[order], d128=d128[order],
                       g=g[order], w=w[order], idx16=idx16[order]))

    gmax = max(GROUP_SIZES)
    cnt = np.zeros((n_types, N_CORES, NG, NW), np.int64)
    cnt_s = np.zeros((n_types, N_CORES, NG, NW, gmax), np.int64)
    for t in range(n_types):
        e = ed[t]
        s_loc = e["s"] - grp_base[e["g"]]
        np.add.at(cnt[t], (e["c"], e["g"], e["w"]), 1)
        np.add.at(cnt_s[t], (e["c"], e["g"], e["w"], s_loc), 1)

    caps = np.zeros((NG, NW, n_types), np.int64)
    for g in range(NG):
        for w in range(NW):
            for t in range(n_types):
                caps[g, w, t] = _cdiv(int(cnt[t][:, g, w].max()), 128)
    for g in range(NG):
        for t in range(n_types):
            if caps[g, :, t].sum() == 0:
                caps[g, 0, t] = 1

    chunk_base = np.zeros((NG, NW, n_types), np.int64)
    pos = 0
    for g in range(NG):
        for w in range(NW):
            for t in range(n_types):
                chunk_base[g, w, t] = pos
                pos += int(caps[g, w, t])
    n_chunks = pos

    calls = []  # (window, col0, width)
    for g in range(NG):
        for w in range(NW):
            c0 = int(chunk_base[g, w, 0])
            c1 = int(chunk_base[g, w, n_types - 1] + caps[g, w, n_types - 1])
            c = c0
            while c < c1:
                wd = min(KG, c1 - c)
                calls.append((w, c, wd))
                c += wd

    # per-chunk union (over cores) of spanned local slots
    slots_of_chunk = [set() for _ in range(n_chunks)]
    for g in range(NG):
        gsz = GROUP_SIZES[g]
        for w in range(NW):
            for t in range(n_types):
                Q = int(caps[g, w, t])
                if Q == 0:
                    continue
                base = int(chunk_base[g, w, t])
                for c in range(N_CORES):
                    cum = 0
                    for sl in range(gsz):
                        n = int(cnt_s[t][c, g, w, sl])
                        if n == 0:
                            continue
                        q0, q1 = cum // 128, (cum + n - 1) // 128
                        for q in range(q0, q1 + 1):
                            slots_of_chunk[base + q].add(sl)
                        cum += n

    # coverage injection for (t, s) with no edges anywhere
    covered = np.zeros((n_types, S), bool)
    for g in range(NG):
        for w in range(NW):
            for t in range(n_types):
                base = int(chunk_base[g, w, t])
                for q in range(int(caps[g, w, t])):
                    for sl in slots_of_chunk[base + q]:
                        covered[t, grp_base[g] + sl] = True
    for t in range(n_types):
        for s in range(S):
            if not covered[t, s]:
                g = int(grp_of[s])
                for w in range(NW):
                    if caps[g, w, t] > 0:
                        base = int(chunk_base[g, w, t])
                        slots_of_chunk[base].add(s - int(grp_base[g]))
                        break

    # merge adjacent slots (within a quad) into 256-wide subchunks
    # subs_of_chunk[ci] = [(sl_lo, n_slots)]; cover[(ci, sl)] = (j, sl_lo)
    subs_of_chunk = [[] for _ in range(n_chunks)]
    for ci in range(n_chunks):
        SL = sorted(slots_of_chunk[ci])
        i = 0
        while i < len(SL):
            sl = SL[i]
            if (i + 1 < len(SL) and SL[i + 1] == sl + 1 and sl % 4 < 3):
                subs_of_chunk[ci].append((sl, 2))
                i += 2
            else:
                subs_of_chunk[ci].append((sl, 1))
                i += 1

    # enumerate subs in stream order; pack their 1-2 indicator blocks into
    # batches of NB_BLK blocks (one DVE tensor_tensor per batch); a
    # 2-block sub never straddles a batch boundary
    NB_BLK = 8
    sub_id = {}     # (ci, sl_lo) -> j
    cover = {}      # (ci, sl) -> (j, sl_lo)
    sub_blk = {}    # j -> (batch, off)
    sub_ns = {}     # j -> blocks (1 or 2)
    chunk_cell = [None] * n_chunks
    first_q, last_q = {}, {}   # (t, g, quad) -> j
    j = 0
    batch, fill = 0, 0
    for g in range(NG):
        for w in range(NW):
            for t in range(n_types):
                base = int(chunk_base[g, w, t])
                for q in range(int(caps[g, w, t])):
                    ci = base + q
                    chunk_cell[ci] = (g, w, t)
                    for (sl, ns) in subs_of_chunk[ci]:
                        if fill + ns > NB_BLK:
                            batch += 1
                            fill = 0
                        sub_id[(ci, sl)] = j
                        sub_blk[j] = (batch, fill)
                        sub_ns[j] = ns
                        fill += ns
                        if fill == NB_BLK:
                            batch += 1
                            fill = 0
                        for k in range(ns):
                            cover[(ci, sl + k)] = (j, sl)
                        qkey = (t, g, sl // 4)
                        if qkey not in first_q:
                            first_q[qkey] = j
                        last_q[qkey] = j
                        j += 1
    n_sub = j
    n_batches = batch + (1 if fill else 0)

    # ------------------------------------------------------ per-core tables
    per_core = []
    for c in range(N_CORES):
        flat_idx = np.zeros(n_chunks * 128, np.int16)
        scl = np.full((128, n_batches * NB_BLK), SENT, np.float16)
        for t in range(n_types):
            e = ed[t]
            mask = e["c"] == c
            idx = np.nonzero(mask)[0]
            if len(idx) == 0:
                continue
            gs, ws = e["g"][idx], e["w"][idx]
            sl = e["s"][idx] - grp_base[gs]
            cellkey = gs * NW + ws
            bounds = np.nonzero(np.diff(cellkey))[0] + 1
            starts = np.concatenate([[0], bounds])
            ends = np.concatenate([bounds, [len(idx)]])
            for lo, hi in zip(starts, ends):
                g, w = int(gs[lo]), int(ws[lo])
                base = int(chunk_base[g, w, t])
                p = np.arange(hi - lo)
                eidx = idx[lo:hi]
                flat_idx[base * 128 + p] = e["idx16"][eidx]
                slr = sl[lo:hi]
                rb = np.nonzero(np.diff(slr))[0] + 1
                rst = np.concatenate([[0], rb])
                ren = np.concatenate([rb, [hi - lo]])
                for a, b in zip(rst, ren):
                    s_loc = int(slr[a])
                    for q in range(a // 128, (b - 1) // 128 + 1):
                        pa, pb = max(a, q * 128), min(b, (q + 1) * 128)
                        jj, sl_lo = cover[(base + q, s_loc)]
                        bat, off = sub_blk[jj]
                        rows = np.arange(pa, pb) % 128
                        sel = eidx[pa:pb]
                        val = (e["d128"][sel] + 128.0 * (s_loc - sl_lo)
                               + 128.0 * off).astype(np.float16)
                        for k in range(sub_ns[jj]):
                            scl[rows, bat * NB_BLK + off + k] = val

        gidx = np.zeros((128, n_chunks * 8), np.int16)
        for (w, col0, wd) in calls:
            seg = flat_idx[col0 * 128:(col0 + wd) * 128]
            gidx[:, col0 * 8:(col0 + wd) * 8] = \
                np.tile(seg.reshape(-1, 16).T, (8, 1))

        invb = []
        for t in range(n_types):
            row = np.zeros(S * 128, np.float32)
            row[:ROWS_PER_CORE] = invdeg[t][c * ROWS_PER_CORE:
                                            (c + 1) * ROWS_PER_CORE]
            invb.append(np.ascontiguousarray(
                np.broadcast_to(row, (128, S * 128))))
        per_core.append(dict(gidx=np.ascontiguousarray(gidx), scl=scl,
                             invb=invb))

    return dict(caps=caps, chunk_base=chunk_base, n_chunks=n_chunks,
                n_sub=n_sub, calls=calls, subs_of_chunk=subs_of_chunk,
                sub_id=sub_id, sub_blk=sub_blk, sub_ns=sub_ns,
                n_batches=n_batches, nb_blk=NB_BLK,
                chunk_cell=chunk_cell, first_q=first_q, last_q=last_q,
                grp_base=grp_base, per_core=per_core)


# ------------------------------------------------------------ bass program ---

def _build_program(rt, n_nodes, n_cores, reps=1):
    import os
    import concourse.bacc as bacc
    from concourse import mybir, tile, library_config

    mode = os.environ.get("KMODE", "full")  # full | gather | noind | nomm
    NG = len(GROUP_SIZES)
    n_types = 2
    caps, chunk_base = rt["caps"], rt["chunk_base"]
    n_chunks, n_sub = rt["n_chunks"], rt["n_sub"]
    calls, subs_of_chunk = rt["calls"], rt["subs_of_chunk"]
    sub_id, sub_blk, sub_ns = rt["sub_id"], rt["sub_blk"], rt["sub_ns"]
    n_batches, NB_BLK = rt["n_batches"], rt["nb_blk"]
    first_q, last_q = rt["first_q"], rt["last_q"]
    grp_base = rt["grp_base"]

    nc = bacc.Bacc("TRN2", target_bir_lowering=False, debug=False,
                   num_devices=n_cores, dynamic_dma_scratch_size=SCRATCH,
                   num_swdge_queues=NQ)
    dt = mybir.dt

    hpk = nc.dram_tensor("hpk", [n_nodes, 128], dt.float16,
                         kind="ExternalInput").ap()
    gidx_d = nc.dram_tensor("gidx", [128, n_chunks * 8], dt.int16,
                            kind="ExternalInput").ap()
    scl_d = nc.dram_tensor("scl", [128, n_batches * NB_BLK], dt.float16,
                           kind="ExternalInput").ap()
    invb_d = [nc.dram_tensor(f"invb{t}", [128, S * 128], dt.float32,
                             kind="ExternalInput").ap()
              for t in range(n_types)]
    hot_d = nc.dram_tensor("hot", [128, S * 128], dt.float32r,
                           kind="ExternalInput").ap()
    w_d = [nc.dram_tensor(w, [128, 128], dt.float32r,
                          kind="ExternalInput").ap()
           for w in ("w1t", "w2t", "wlt")]
    blc_d = nc.dram_tensor("blc", [128, 1], dt.float32,
                           kind="ExternalInput").ap()
    iota_d = nc.dram_tensor("iota", [128, NB_BLK * 128], dt.float16,
                            kind="ExternalInput").ap()
    outT_d = nc.dram_tensor("outT", [128, S * 128], dt.float32,
                            kind="ExternalOutput").ap()

    call_of_chunk = {}
    for k, (w, col0, wd) in enumerate(calls):
        for ci in range(col0, col0 + wd):
            call_of_chunk[ci] = k

    with tile.TileContext(nc) as tc:
        with (
            tc.tile_pool(name="const", bufs=1) as const_p,
            tc.tile_pool(name="gpool", bufs=6) as gpool,
            tc.tile_pool(name="ind", bufs=8) as ind_p,
            tc.tile_pool(name="mt", bufs=2) as mt_p,
            tc.tile_pool(name="invb", bufs=2) as invb_p,
            tc.tile_pool(name="hot", bufs=2) as hot_p,
            tc.tile_pool(name="ostage", bufs=2) as o_p,
            tc.tile_pool(name="psT", bufs=1, space="PSUM") as psT_p,
            tc.tile_pool(name="pso", bufs=2, space="PSUM") as pso_p,
        ):
            nc.gpsimd.load_library(library_config.mlp)
            gidx_s = const_p.tile([128, n_chunks * 8], dt.int16, name="gidx_s")
            nc.sync.dma_start(out=gidx_s[:], in_=gidx_d[:, :])
            scl_s = const_p.tile([128, n_batches * NB_BLK], dt.float16,
                                 name="scl_s")
            nc.sync.dma_start(out=scl_s[:], in_=scl_d[:, :])
            w_s = []
            for i, wd_ in enumerate(w_d):
                wt = const_p.tile([128, 128], dt.float32r, tag=f"w{i}",
                                  name=f"ws{i}")
                nc.sync.dma_start(out=wt[:], in_=wd_[:, :])
                w_s.append(wt)
            blc_s = const_p.tile([128, 1], dt.float32, name="blc_s")
            nc.sync.dma_start(out=blc_s[:], in_=blc_d[:, :])
            iota_s = const_p.tile([128, NB_BLK * 128], dt.float16,
                                  name="iota_s")
            nc.sync.dma_start(out=iota_s[:], in_=iota_d[:, :])

            f32r = dt.float32r
            relu = mybir.ActivationFunctionType.Relu
            iseq = mybir.AluOpType.is_equal
            mult = mybir.AluOpType.mult

            for rep in range(reps):
                call_ctr = 0
                ind_tiles = {}  # batch -> tile (current rep, rolling)
                for g in range(NG):
                    gsz = GROUP_SIZES[g]
                    gb = int(grp_base[g])
                    # inv tables for this group (overlaps with gathers)
                    invb_s = []
                    for t in range(n_types):
                        iv = invb_p.tile([128, gsz * 128], dt.float32,
                                         tag=f"invb{t}", name=f"invb{t}")
                        nc.sync.dma_start(
                            out=iv[:],
                            in_=invb_d[t][:, gb * 128:(gb + gsz) * 128])
                        invb_s.append(iv)
                    ps = {}  # (t, quad) -> [128, 512] psum bank tile
                    g_tile = None
                    cur_call = -1
                    for w in range(NW):
                        c0 = int(chunk_base[g, w, 0])
                        c1 = int(chunk_base[g, w, n_types - 1]
                                 + caps[g, w, n_types - 1])
                        for ci in range(c0, c1):
                            k = call_of_chunk[ci]
                            if k != cur_call:
                                cur_call = k
                                wn, col0, wd = calls[k]
                                b0 = wn * WBASE
                                b1 = min(b0 + WBASE, n_nodes)
                                qn = call_ctr % NQ
                                call_ctr += 1
                                g_tile = gpool.tile(
                                    [128, KG, 128], dt.float16,
                                    tag=f"g{qn}", name="g")
                                nc.gpsimd.dma_gather(
                                    g_tile[:, :wd, :], hpk[b0:b1, :],
                                    gidx_s[:, col0 * 8:(col0 + wd) * 8],
                                    128 * wd, 128 * wd, 128,
                                    single_packet=SP, queue_num=qn)
                            jj = ci - calls[k][1]
                            gg, ww, t = rt["chunk_cell"][ci]
                            if mode == "gather":
                                continue
                            for (sl, ns) in subs_of_chunk[ci]:
                                j = sub_id[(ci, sl)]
                                wide = ns * 128
                                bat, off = sub_blk[j]
                                if bat not in ind_tiles and (
                                        mode != "noind" or not ind_tiles
                                        or bat % 8 == 0):
                                    ind = ind_p.tile(
                                        [128, NB_BLK * 128], dt.float16,
                                        tag="ind", name="ind")
                                    from concourse import bass as _bass
                                    slc = scl_s[:, bat * NB_BLK:
                                                (bat + 1) * NB_BLK]
                                    bc = _bass.AP(
                                        slc.tensor, slc.offset,
                                        slc.ap + [[0, 128]])
                                    nc.vector.tensor_tensor(
                                        out=ind[:], in0=iota_s[:],
                                        in1=bc, op=iseq)
                                    ind_tiles = {bat: ind}
                                elif bat not in ind_tiles:
                                    ind_tiles = {bat: ind}
                                ind = ind_tiles[bat]
                                if mode == "nomm":
                                    continue
                                qd = sl // 4
                                if (t, qd) not in ps:
                                    ps[(t, qd)] = psT_p.tile(
                                        [128, 512], dt.float32,
                                        tag=f"ps{t}_{qd}", name=f"ps{t}_{qd}")
                                co = (sl % 4) * 128
                                st = first_q[(t, g, qd)] == j
                                sp = last_q[(t, g, qd)] == j
                                nc.tensor.matmul(
                                    out=ps[(t, qd)][:, co:co + wide],
                                    lhsT=g_tile[:, jj, :],
                                    rhs=ind[:, off * 128:off * 128 + wide],
                                    start=st, stop=sp)

                    # ---------------- finalize group: weight matmuls + out
                    if mode in ("gather", "nomm"):
                        continue
                    for pl in range(gsz // 2):
                        q2 = (gb + 2 * pl) // 2
                        mts = []
                        for t in range(n_types):
                            mt = mt_p.tile([128, 256], f32r, tag=f"mt{t}",
                                           name=f"mt{t}")
                            qd, co = pl // 2, (pl % 2) * 256
                            nc.vector.tensor_tensor(
                                out=mt[:], in0=ps[(t, qd)][:, co:co + 256],
                                in1=invb_s[t][:, pl * 256:(pl + 1) * 256],
                                op=mult)
                            mts.append(mt)
                        hot_t = hot_p.tile([128, 256], f32r, tag="hot",
                                           name="hot_t")
                        nc.sync.dma_start(
                            out=hot_t[:],
                            in_=hot_d[:, q2 * 256:(q2 + 1) * 256])
                        pso = pso_p.tile([128, 256], dt.float32, tag="pso",
                                         name="pso")
                        nc.tensor.matmul(out=pso[:], lhsT=w_s[0][:],
                                         rhs=mts[0][:], start=True,
                                         stop=False)
                        nc.tensor.matmul(out=pso[:], lhsT=w_s[1][:],
                                         rhs=mts[1][:], start=False,
                                         stop=False)
                        nc.tensor.matmul(out=pso[:], lhsT=w_s[2][:],
                                         rhs=hot_t[:], start=False,
                                         stop=True)
                        ot = o_p.tile([128, 256], dt.float32, tag="ot",
                                      name="ot")
                        nc.scalar.activation(out=ot[:], in_=pso[:],
                                             func=relu, bias=blc_s[:, 0:1])
                        nc.sync.dma_start(
                            out=outT_d[:, q2 * 256:(q2 + 1) * 256],
                            in_=ot[:])

    nc.compile()
    return nc


# ------------------------------------------------------------------ driver ---

def _prepare(h, src1, dst1, src2, dst2, W1, W2, Wl, bl,
             rows_per_core=ROWS_PER_CORE, n_cores=N_CORES):
    h = np.asarray(h, np.float32)
    bl = np.asarray(bl, np.float32)
    srcs = [np.asarray(src1), np.asarray(src2)]
    dsts = [np.asarray(dst1), np.asarray(dst2)]
    rt = _route(srcs, dsts)

    hpk = np.ascontiguousarray(h.astype(np.float16))  # [N, 128] fp16

    w1t = (0.5 * np.asarray(W1, np.float32).T).copy()
    w2t = (0.5 * np.asarray(W2, np.float32).T).copy()
    wlt = np.asarray(Wl, np.float32).T.copy()
    blc = bl.reshape(128, 1).copy()
    W_IND = rt["nb_blk"] * 128
    iota = np.broadcast_to(np.arange(W_IND, dtype=np.float32),
                           (128, W_IND))
    iota = np.ascontiguousarray(iota.astype(np.float16))

    in_maps = []
    for c in range(n_cores):
        pc = rt["per_core"][c]
        rows = h[c * rows_per_core:(c + 1) * rows_per_core]
        pad = S * 128 - rows.shape[0]
        rows = np.pad(rows, ((0, pad), (0, 0)))
        hot = np.ascontiguousarray(rows.T)  # [128, S*128]
        in_maps.append(dict(
            hpk=hpk, gidx=pc["gidx"], scl=pc["scl"],
            invb0=pc["invb"][0], invb1=pc["invb"][1], hot=hot,
            w1t=w1t, w2t=w2t, wlt=wlt, blc=blc, iota=iota,
        ))
    return rt, in_maps


def _postprocess(results, rt, rows_per_core=ROWS_PER_CORE, n_cores=N_CORES):
    n_nodes = rows_per_core * n_cores
    out = np.empty((n_nodes, HIDDEN), np.float32)
    for c in range(n_cores):
        outT = results[c]["outT"]  # [128, S*128]
        out[c * rows_per_core:(c + 1) * rows_per_core] = \
            outT[:, :rows_per_core].T
    return out


def kernel(h, src1, dst1, src2, dst2, W1, W2, Wl, bl, **kw):
    from concourse import bass_utils
    rt, in_maps = _prepare(h, src1, dst1, src2, dst2, W1, W2, Wl, bl)
    nc = _build_program(rt, N_NODES, N_CORES)
    res = bass_utils.run_bass_kernel_spmd(
        nc, in_maps, core_ids=list(range(N_CORES)))
    return _postprocess(res.results, rt)
